# revision 1
# baseline (speedup 1.0000x reference)
"""Cross-attention Trainium2 kernel (Bass/Tile), data-parallel over batch on 8 cores.

Reference computation per batch b (C=256, CR=64, N=H*W=4096):
    Q = Wq @ src          [CR, N]
    K = Wk @ gui          [CR, N]
    V = Wv @ gui + bv     [C, N]
    energy[n, m] = sum_q Q[q, n] K[q, m]
    attn = softmax_m(energy)
    out = gamma * (V @ attn^T) + src

Kernel strategy (per core, one batch item):
    - compute energy TRANSPOSED: eT[m, n] = sum_q K[q, m] Q[q, n] so the
      unnormalized attention tiles come out of the PE in exactly the [m, n]
      orientation the V @ attn^T matmul needs as its moving operand.
    - exp on ScalarE with bias -2 (softmax-invariant; keeps exp outputs in
      fp8-e4m3 range). Energy m-chunk PAIRS share one 2-bank PSUM tile so a
      single ACT instruction drains 1024 elems (amortizes ACT fixed costs)
      straight into the [K, 2, N] fp8 layout DoubleRow matmuls consume.
    - attention-value + row-sum matmuls run in fp8 DoubleRow perf mode
      (2 k-tiles per pass, double PE throughput); V^T stationary is fp8.
    - row sums via a ones-matmul (sum over the partition dim on the PE),
      replicated across all 128 partitions so the final normalization is a
      plain elementwise multiply.
    - normalization, gamma and residual folded into the PSUM->SBUF drain.
    - matmul operands in bf16 (full PE rate + fast weight load); every
      operand already passes through a DVE/ACT drain, so the conversions are
      free. PSUM accumulation stays fp32. The residual path keeps the
      original fp32 `source`.
    - Q/K live duplicated on partitions 0-63 / 64-127 so the K=64 energy
      matmuls run pairwise-concurrent in the two PE row-group halves.
"""

from contextlib import ExitStack

import numpy as np

import concourse.bacc as bacc
import concourse.bass as bass
import concourse.mybir as mybir
import concourse.tile as tile
from concourse.bass_utils import run_bass_kernel_spmd
from concourse.masks import make_identity

B, C, H, W = 8, 256, 64, 64
N = H * W            # 4096 pixels
CR = 64              # reduced channels for Q/K
N_CORES = 8
NT = 512             # n-chunk (query) tile
NCH = N // NT        # 8
MT = 128             # m-chunk (key) tile: PE output partition max
MCH = N // MT        # 32
CCH = C // 128       # 2 channel chunks

F32 = mybir.dt.float32
BF16 = mybir.dt.bfloat16
FP8 = mybir.dt.float8e4          # e4m3, max 448
EXP = mybir.ActivationFunctionType.Exp
DR = mybir.MatmulPerfMode.DoubleRow

ts = bass.ts

ROW_TILE = True  # pairwise-concurrent energy matmuls in PE row-group halves
EBIAS = -2.0     # exp(e + EBIAS): keeps fp8 attn weights < 448 (softmax-invariant)


def build_kernel(loop=1):
    """Build + compile the single-core program (SPMD across 8 cores).

    loop > 1 unrolls the whole kernel body that many times in one NEFF; used
    by test.py to measure marginal (steady-state) HW time per execution.
    """
    nc = bacc.Bacc("TRN2", target_bir_lowering=False, debug=False)

    src_d = nc.dram_tensor("source", [C, N], F32, kind="ExternalInput").ap()
    gui_d = nc.dram_tensor("guidance", [C, N], F32, kind="ExternalInput").ap()
    wq_d = nc.dram_tensor("Wq", [CR, C], F32, kind="ExternalInput").ap()
    wk_d = nc.dram_tensor("Wk", [CR, C], F32, kind="ExternalInput").ap()
    wv_d = nc.dram_tensor("Wv", [C, C], F32, kind="ExternalInput").ap()
    bv_d = nc.dram_tensor("bv", [C], F32, kind="ExternalInput").ap()
    g_d = nc.dram_tensor("gamma", [1], F32, kind="ExternalInput").ap()
    out_d = nc.dram_tensor("out", [C, N], F32, kind="ExternalOutput").ap()

    with tile.TileContext(nc) as tc:
        for it in range(loop):
            with ExitStack() as ctx:
                _body(ctx, tc, src_d, gui_d, wq_d, wk_d, wv_d, bv_d, g_d,
                      out_d, sfx=f"_{it}")
    nc.compile()
    return nc


def _body(ctx, tc, src_d, gui_d, wq_d, wk_d, wv_d, bv_d, g_d, out_d, sfx=""):
    nc = tc.nc

    consts = ctx.enter_context(tc.tile_pool(name="consts" + sfx, bufs=1))
    big = ctx.enter_context(tc.tile_pool(name="big" + sfx, bufs=1))

    # ---- persistent SBUF tensors ----
    src_sb = big.tile([128, CCH, N], F32)    # fp32: residual + Q-proj moving
    gui8 = big.tile([128, CCH, N], FP8)      # fp8 DoubleRow operand for K/V
    # Q/K with q duplicated onto partitions 64..127 for PE row-tiling.
    QQ = big.tile([128, N], BF16)
    KK = big.tile([128, N], BF16)
    # V^T in fp8, m-chunk pairs interleaved for DoubleRow: [m%128, pair, ktile, c]
    VT2 = big.tile([128, MCH // 2, 2, C], FP8)

    # ---- weights / constants ----
    wq_sb = consts.tile([CR, C], F32)
    wk_sb = consts.tile([CR, C], F32)
    wv_sb = consts.tile([128, CCH, C], F32)  # [c%128, c//128, ch]
    bv_sb = consts.tile([1, C], BF16)
    g128 = consts.tile([128, 1], F32)
    ebias = consts.tile([128, 1], F32)       # exp bias (softmax-invariant)
    ones = consts.tile([1, 128], BF16)       # bias-row matmul stationary
    ones8 = consts.tile([128, 2, 128], FP8)  # DoubleRow row-sum stationary
    ident = consts.tile([128, 128], F32)
    scr = consts.tile([1, 1], F32)

    nc.sync.dma_start(out=wq_sb[:], in_=wq_d)
    nc.sync.dma_start(out=wk_sb[:], in_=wk_d)
    wv_r = wv_d.rearrange("(t p) c -> t p c", p=128)
    for t in range(CCH):
        nc.sync.dma_start(out=wv_sb[:, t, :], in_=wv_r[t])
    bv_f = consts.tile([1, C], F32)
    nc.sync.dma_start(out=bv_f[:], in_=bv_d.unsqueeze(0))
    nc.vector.tensor_copy(bv_sb[:], bv_f[:])
    nc.sync.dma_start(out=g128[:], in_=g_d.to_broadcast([128, 1]))
    nc.vector.memset(ones[:], 1.0)
    nc.vector.memset(ones8[:], 1.0)
    nc.vector.memset(ebias[:], EBIAS)
    # warm the ACT exp table while DMAs run (Copy/Exp share one table set)
    nc.scalar.activation(scr[:], ebias[0:1, :], EXP)
    make_identity(nc, ident[:])

    # ---- load activations: gui first (K/V projections gate the main loop
    # less than Q does; Q reads src in fp32 directly, no conversion step) ----
    src_r = src_d.rearrange("(t p) n -> t p n", p=128)
    gui_r = gui_d.rearrange("(t p) n -> t p n", p=128)
    with tc.tile_pool(name="stage" + sfx, bufs=1) as stage:
        gui_f = stage.tile([128, CCH, N], F32)
        for t in range(CCH):
            nc.sync.dma_start(out=gui_f[:, t, :], in_=gui_r[t])
        for t in range(CCH):
            nc.sync.dma_start(out=src_sb[:, t, :], in_=src_r[t])

        # ---- transpose weights on the PE (fp32 has no DMA transpose) ----
        # wqt_f: fp32 [c-chunk, q dup]; wkt8/wvt8: fp8 DoubleRow stationaries
        wqt_f = consts.tile([128, CCH, 128], F32)
        wkt8 = consts.tile([128, CCH, 128], FP8)
        wvt8 = consts.tile([128, CCH, C], FP8)

        with tc.tile_pool(name="tp_psum" + sfx, bufs=2,
                          space=bass.MemorySpace.PSUM) as tpp:
            for t in range(CCH):
                p = tpp.tile([128, CR], F32, tag="tp")
                nc.tensor.transpose(p[:], wq_sb[:, ts(t, 128)], ident[:CR, :CR])
                nc.vector.tensor_copy(wqt_f[:, t, 0:CR], p[:])
                nc.vector.tensor_copy(wqt_f[:, t, CR:128], p[:])
                p = tpp.tile([128, CR], F32, tag="tp")
                nc.tensor.transpose(p[:], wk_sb[:, ts(t, 128)], ident[:CR, :CR])
                nc.vector.tensor_copy(wkt8[:, t, 0:CR], p[:])
                nc.vector.tensor_copy(wkt8[:, t, CR:128], p[:])
                for j in range(CCH):
                    # wvt8[:, t, j*128:+128] = Wv[j*128:+128, t*128:+128]^T
                    p = tpp.tile([128, 128], F32, tag="tp")
                    nc.tensor.transpose(p[:], wv_sb[:, j, ts(t, 128)], ident[:])
                    nc.vector.tensor_copy(wvt8[:, t, ts(j, 128)], p[:])

        # fp8 conversion per chunk, split across ACT / DVE
        nc.scalar.copy(gui8[:, 0, :], gui_f[:, 0, :])
        nc.vector.tensor_copy(gui8[:, 1, :], gui_f[:, 1, :])

        # ---- K/V projections (fp8 DoubleRow, both c-chunks in one pass);
        # hidden under the src DMA.  Q(i=0) in fp32 straight off src_sb. ----
        with tc.tile_pool(name="proj_psum" + sfx, bufs=4,
                          space=bass.MemorySpace.PSUM) as pp:
            for i in range(NCH):
                kp = pp.tile([128, NT], F32, tag="qk")
                nc.tensor.matmul(kp[:], wkt8[:, :, :], gui8[:, :, ts(i, NT)],
                                 perf_mode=DR)
                if i % 2 == 0:
                    nc.scalar.copy(KK[:, ts(i, NT)], kp[:])
                else:
                    nc.vector.tensor_copy(KK[:, ts(i, NT)], kp[:])
            for j in range(MCH):
                vp = pp.tile([128, C], F32, tag="v")
                # bias row via K=1 ones-matmul: vp[m, c] = bv[c]
                nc.tensor.matmul(vp[:], ones[0:1, :], bv_sb[:],
                                 start=True, stop=False)
                nc.tensor.matmul(vp[:], gui8[:, :, ts(j, MT)], wvt8[:, :, :],
                                 start=False, stop=True, perf_mode=DR)
                if j % 2 == 0:
                    nc.scalar.copy(VT2[:, j // 2, j % 2, :], vp[:])
                else:
                    nc.vector.tensor_copy(VT2[:, j // 2, j % 2, :], vp[:])
            qp = pp.tile([128, NT], F32, tag="qk")
            for t in range(CCH):
                nc.tensor.matmul(qp[:], wqt_f[:, t, :], src_sb[:, t, 0:NT],
                                 start=(t == 0), stop=(t == CCH - 1))
            nc.vector.tensor_copy(QQ[:, 0:NT], qp[:])

    # ---- attention main loop ----
    # PSUM budget: e_ps 2x2-bank + o_ps 2 + s_ps 1 + q_ps 1 = 8 banks.
    e_ps = ctx.enter_context(
        tc.tile_pool(name="e_psum" + sfx, bufs=2, space=bass.MemorySpace.PSUM))
    o_ps = ctx.enter_context(
        tc.tile_pool(name="o_psum" + sfx, bufs=2, space=bass.MemorySpace.PSUM))
    s_ps = ctx.enter_context(
        tc.tile_pool(name="s_psum" + sfx, bufs=1, space=bass.MemorySpace.PSUM))
    q_ps = ctx.enter_context(
        tc.tile_pool(name="q_psum" + sfx, bufs=1, space=bass.MemorySpace.PSUM))
    e_sb = ctx.enter_context(tc.tile_pool(name="e_sb" + sfx, bufs=4))
    fin = ctx.enter_context(tc.tile_pool(name="fin" + sfx, bufs=2))
    o_sb = ctx.enter_context(tc.tile_pool(name="o_sb" + sfx, bufs=4))

    out_r = out_d.rearrange("(t p) n -> t p n", p=128)
    NPAIR = MCH // 2  # 16 m-chunk pairs per query tile

    for i in range(NCH):
        o0 = o_ps.tile([128, NT], F32, tag="o")
        o1 = o_ps.tile([128, NT], F32, tag="o")
        sm = s_ps.tile([128, NT], F32, tag="s")

        def energy_pair(jj):
            # two m-chunks into one 2-bank PSUM tile -> single ACT exp drain
            ep = e_ps.tile([128, 2, NT], F32, tag="e")
            for h in range(2):
                j = 2 * jj + h
                b0 = CR * (j % 2) if ROW_TILE else 0
                nc.tensor.matmul(ep[:, h, :], KK[b0:b0 + CR, ts(j, MT)],
                                 QQ[b0:b0 + CR, ts(i, NT)],
                                 start=True, stop=True, tile_position=(b0, 0))
            return ep

        ep = energy_pair(0)
        for jj in range(NPAIR):
            ee = e_sb.tile([128, 2, NT], FP8, tag="ee")
            nc.scalar.activation(ee[:], ep[:], EXP, bias=ebias[:])
            if jj + 1 < NPAIR:
                ep = energy_pair(jj + 1)  # keep PE one pair ahead of ACT
            if jj == 0 and i + 1 < NCH:
                # project Q for the next query tile in-loop (fp32 moving off
                # src_sb: no conversion step gates the first iteration)
                qp = q_ps.tile([128, NT], F32, tag="q")
                for t in range(CCH):
                    nc.tensor.matmul(qp[:], wqt_f[:, t, :],
                                     src_sb[:, t, ts(i + 1, NT)],
                                     start=(t == 0), stop=(t == CCH - 1))
                nc.vector.tensor_copy(QQ[:, ts(i + 1, NT)], qp[:])
            first, last = jj == 0, jj == NPAIR - 1
            nc.tensor.matmul(o0[:], VT2[:, jj, :, 0:128], ee[:],
                             start=first, stop=last, perf_mode=DR)
            nc.tensor.matmul(o1[:], VT2[:, jj, :, 128:256], ee[:],
                             start=first, stop=last, perf_mode=DR)
            nc.tensor.matmul(sm[:], ones8[:], ee[:], start=first, stop=last,
                             perf_mode=DR)

        # out = o * (gamma / sum) + src; residual add on the idle Pool engine
        rsg = fin.tile([128, NT], F32, tag="rsg")
        nc.vector.reciprocal_approx_fast(rsg[:], sm[:])
        nc.vector.tensor_scalar_mul(rsg[:], rsg[:], g128[:])
        for t, op in enumerate((o0, o1)):
            ot = o_sb.tile([128, NT], F32, tag="ot")
            nc.vector.tensor_mul(ot[:], op[:], rsg[:])
            nc.gpsimd.tensor_add(ot[:], ot[:], src_sb[:, t, ts(i, NT)])
            nc.sync.dma_start(out=out_r[t][:, ts(i, NT)], in_=ot[:])


_NC_CACHE = []


def _get_nc():
    if not _NC_CACHE:
        _NC_CACHE.append(build_kernel())
    return _NC_CACHE[0]


def make_in_maps(**inputs):
    f = lambda a: np.ascontiguousarray(np.asarray(a, dtype=np.float32))
    src = f(inputs["source"]).reshape(B, C, N)
    gui = f(inputs["guidance"]).reshape(B, C, N)
    shared = {
        "Wq": f(inputs["Wq"]),
        "Wk": f(inputs["Wk"]),
        "Wv": f(inputs["Wv"]),
        "bv": f(inputs["bv"]),
        "gamma": f(inputs["gamma"]),
    }
    return [dict(source=src[b], guidance=gui[b], **shared) for b in range(B)]


def kernel(**inputs) -> np.ndarray:
    nc = _get_nc()
    res = run_bass_kernel_spmd(nc, make_in_maps(**inputs),
                               core_ids=list(range(N_CORES)))
    out = np.stack([res.results[b]["out"] for b in range(B)])
    return out.reshape(B, C, H, W).astype(np.float32)



# revision 3
# speedup vs baseline: 18.7070x; 18.7070x over previous
"""Cross-attention Trainium2 kernel (Bass/Tile), data-parallel over batch on 8 cores.

Reference computation per batch b (C=256, CR=64, N=H*W=4096):
    Q = Wq @ src          [CR, N]
    K = Wk @ gui          [CR, N]
    V = Wv @ gui + bv     [C, N]
    energy[n, m] = sum_q Q[q, n] K[q, m]
    attn = softmax_m(energy)
    out = gamma * (V @ attn^T) + src

Kernel strategy (per core, one batch item). The wall-clock floor is the
Scalar-engine exp stream over the N*N energy matrix (~16.8M elems); every
other engine is organized to keep that stream dense:
    - energy computed TRANSPOSED: eT[m, n] = sum_q K[q, m] Q[q, n] so the
      unnormalized attention tiles leave the PE in the [m, n] orientation the
      V @ attn^T matmul consumes as its moving operand.
    - exp on ScalarE with bias -2 (softmax-invariant; keeps exp outputs in
      fp8-e4m3 range). Energy m-chunk PAIRS share one 2-bank PSUM tile so a
      single ACT instruction drains 1024 elems straight into the [K, 2, N]
      fp8 layout DoubleRow matmuls consume. ACT does nothing else in the
      main loop.
    - attention-value + row-sum matmuls in fp8 DoubleRow perf mode; V^T
      stationary is fp8 with gamma and bv FOLDED IN at projection time
      (V' = gamma*V + gamma*bv via one scalar_tensor_tensor drain), so the
      final normalization is out = o * (1/rowsum) + src with no gamma step.
    - row sums via a ones-matmul (partition-dim reduction on the PE),
      replicated across all 128 partitions so normalization is a plain
      elementwise multiply.
    - Q projection uses float32r (TF32-class PE mode): full-rate rows at
      >=256 moving width, straight off the fp32 src_sb residual copy via
      bitcast — no conversion pass, 4x faster than fp32 matmul.
    - inputs DMA'd in 1024-column slices, guidance first, so K/V projections
      and the first exp start ~12us in instead of waiting ~23us for whole
      tensors; all projection drains go to DVE/Pool (never ACT).
    - Q/K live duplicated on partitions 0-63 / 64-127 so the K=64 energy
      matmuls run pairwise in the two PE row-group halves (tile_position).
"""

from contextlib import ExitStack

import numpy as np

import concourse.bacc as bacc
import concourse.bass as bass
import concourse.mybir as mybir
import concourse.tile as tile
from concourse.bass_utils import run_bass_kernel_spmd
from concourse.masks import make_identity

B, C, H, W = 8, 256, 64, 64
N = H * W            # 4096 pixels
CR = 64              # reduced channels for Q/K
N_CORES = 8
NT = 512             # n-chunk (query) tile
NCH = N // NT        # 8
MT = 128             # m-chunk (key) tile: PE output partition max
MCH = N // MT        # 32
CCH = C // 128       # 2 channel chunks
SL = 1024            # DMA column-slice width
NSL = N // SL        # 4

F32 = mybir.dt.float32
F32R = mybir.dt.float32r
BF16 = mybir.dt.bfloat16
FP8 = mybir.dt.float8e4          # e4m3, max 448
EXP = mybir.ActivationFunctionType.Exp
DR = mybir.MatmulPerfMode.DoubleRow
MUL = mybir.AluOpType.mult
ADD = mybir.AluOpType.add

ts = bass.ts

EBIAS = -2.0     # exp(e + EBIAS): keeps fp8 attn weights < 448 (softmax-invariant)


def build_kernel(loop=1):
    """Build + compile the single-core program (SPMD across 8 cores).

    loop > 1 unrolls the whole kernel body that many times in one NEFF; used
    by test.py to measure marginal (steady-state) HW time per execution.
    """
    nc = bacc.Bacc("TRN2", target_bir_lowering=False, debug=False)

    src_d = nc.dram_tensor("source", [C, N], F32, kind="ExternalInput").ap()
    gui_d = nc.dram_tensor("guidance", [C, N], F32, kind="ExternalInput").ap()
    wq_d = nc.dram_tensor("Wq", [CR, C], F32, kind="ExternalInput").ap()
    wk_d = nc.dram_tensor("Wk", [CR, C], F32, kind="ExternalInput").ap()
    wv_d = nc.dram_tensor("Wv", [C, C], F32, kind="ExternalInput").ap()
    bv_d = nc.dram_tensor("bv", [C], F32, kind="ExternalInput").ap()
    g_d = nc.dram_tensor("gamma", [1], F32, kind="ExternalInput").ap()
    out_d = nc.dram_tensor("out", [C, N], F32, kind="ExternalOutput").ap()

    with tile.TileContext(nc) as tc:
        for it in range(loop):
            with ExitStack() as ctx:
                _body(ctx, tc, src_d, gui_d, wq_d, wk_d, wv_d, bv_d, g_d,
                      out_d, sfx=f"_{it}")
    nc.compile()
    return nc


def _body(ctx, tc, src_d, gui_d, wq_d, wk_d, wv_d, bv_d, g_d, out_d, sfx=""):
    nc = tc.nc

    consts = ctx.enter_context(tc.tile_pool(name="consts" + sfx, bufs=1))
    big = ctx.enter_context(tc.tile_pool(name="big" + sfx, bufs=1))

    # ---- persistent SBUF tensors ----
    src_sb = big.tile([128, CCH, N], F32)    # fp32: residual + Q-proj moving
    gui8 = big.tile([128, CCH, N], FP8)      # fp8 DoubleRow operand for K/V
    # Q/K with q duplicated onto partitions 64..127 for PE row-tiling.
    QQ = big.tile([128, N], BF16)
    KK = big.tile([128, N], BF16)
    # gamma*V^T (+bias) in fp8, m-chunk pairs interleaved for DoubleRow.
    VT2 = big.tile([128, MCH // 2, 2, C], FP8)

    # ---- weights / constants ----
    wq_sb = consts.tile([CR, C], F32)
    wk_sb = consts.tile([CR, C], F32)
    wv_sb = consts.tile([128, CCH, C], F32)  # [c%128, c//128, ch]
    bv_f = consts.tile([128, C], F32)        # bv broadcast to all partitions
    bvg = consts.tile([128, C], F32)         # gamma * bv
    g128 = consts.tile([128, 1], F32)
    ebias = consts.tile([128, 1], F32)       # exp bias (softmax-invariant)
    ones8 = consts.tile([128, 2, 128], FP8)  # DoubleRow row-sum stationary
    ident = consts.tile([128, 128], F32)
    scr = consts.tile([1, 1], F32)
    # transposed weights: wqt fp32 (bitcast fp32r at use); wkt8/wvt8 fp8 DR
    wqt_f = consts.tile([128, CCH, 128], F32)
    wkt8 = consts.tile([128, CCH, 128], FP8)
    wvt8 = consts.tile([128, CCH, C], FP8)

    # ---- small weight DMAs first (short HWDGE queue ahead of the big loads)
    nc.sync.dma_start(out=wq_sb[:], in_=wq_d)
    nc.sync.dma_start(out=wk_sb[:], in_=wk_d)
    nc.sync.dma_start(out=wv_sb[:], in_=wv_d.rearrange("(t p) c -> p t c", p=128))
    nc.sync.dma_start(out=bv_f[:], in_=bv_d.unsqueeze(0).to_broadcast([128, C]))
    nc.sync.dma_start(out=g128[:], in_=g_d.to_broadcast([128, 1]))

    # ---- input DMAs in column slices; guidance has priority (it gates the
    # K/V projections feeding the exp stream), then src slice 0 for Q tile 0.
    gui_r = gui_d.rearrange("(t p) n -> t p n", p=128)
    src_r = src_d.rearrange("(t p) n -> t p n", p=128)
    stage = ctx.enter_context(tc.tile_pool(name="stage" + sfx, bufs=1))
    gst = [stage.tile([128, CCH, SL], F32, tag=f"g{s}", name=f"gst{s}" + sfx)
           for s in range(NSL)]
    for s in range(2):
        for t in range(CCH):
            nc.sync.dma_start(out=gst[s][:, t, :], in_=gui_r[t][:, ts(s, SL)])
    for t in range(CCH):
        nc.sync.dma_start(out=src_sb[:, t, ts(0, SL)], in_=src_r[t][:, ts(0, SL)])
    for s in range(2, NSL):
        for t in range(CCH):
            nc.sync.dma_start(out=gst[s][:, t, :], in_=gui_r[t][:, ts(s, SL)])
    for s in range(1, NSL):
        for t in range(CCH):
            nc.sync.dma_start(out=src_sb[:, t, ts(s, SL)],
                              in_=src_r[t][:, ts(s, SL)])

    nc.vector.memset(ones8[:], 1.0)
    nc.vector.memset(ebias[:], EBIAS)
    # warm the ACT exp table while DMAs run
    nc.scalar.activation(scr[:], ebias[0:1, :], EXP)
    make_identity(nc, ident[:])
    nc.vector.tensor_scalar_mul(bvg[:], bv_f[:], g128[:])

    # ---- transpose weights on the PE (fp32 has no DMA transpose) ----
    with tc.tile_pool(name="tp_psum" + sfx, bufs=2,
                      space=bass.MemorySpace.PSUM) as tpp:
        for t in range(CCH):
            p = tpp.tile([128, CR], F32, tag="tp")
            nc.tensor.transpose(p[:], wq_sb[:, ts(t, 128)], ident[:CR, :CR])
            nc.vector.tensor_copy(wqt_f[:, t, 0:CR], p[:])
            nc.vector.tensor_copy(wqt_f[:, t, CR:128], p[:])
            p = tpp.tile([128, CR], F32, tag="tp")
            nc.tensor.transpose(p[:], wk_sb[:, ts(t, 128)], ident[:CR, :CR])
            nc.vector.tensor_copy(wkt8[:, t, 0:CR], p[:])
            nc.vector.tensor_copy(wkt8[:, t, CR:128], p[:])
            for j in range(CCH):
                # wvt8[:, t, j*128:+128] = Wv[j*128:+128, t*128:+128]^T
                p = tpp.tile([128, 128], F32, tag="tp")
                nc.tensor.transpose(p[:], wv_sb[:, j, ts(t, 128)], ident[:])
                nc.vector.tensor_copy(wvt8[:, t, ts(j, 128)], p[:])

    # ---- gui -> fp8 conversion per slice. ACT is idle pre-loop: it takes
    # slice 0; later slices split across Pool (t=0) and DVE (t=1).
    for t in range(CCH):
        nc.scalar.copy(gui8[:, t, ts(0, SL)], gst[0][:, t, :])
    for s in range(1, NSL):
        nc.gpsimd.tensor_copy(gui8[:, 0, ts(s, SL)], gst[s][:, 0, :])
        nc.vector.tensor_copy(gui8[:, 1, ts(s, SL)], gst[s][:, 1, :])

    # ---- K/V projections for all slices, pipelined through a 4-buf PSUM
    # pool (closed before the main-loop pools open). PE stalls only on the
    # gui slice DMAs; drains alternate DVE/Pool. Q tile 0 projected last.
    with tc.tile_pool(name="proj_psum" + sfx, bufs=4,
                      space=bass.MemorySpace.PSUM) as pp:
        for s in range(NSL):
            for i in range(s * (NCH // NSL), (s + 1) * (NCH // NSL)):
                kp = pp.tile([128, NT], F32, tag="p")
                nc.tensor.matmul(kp[:], wkt8[:, :, :], gui8[:, :, ts(i, NT)],
                                 perf_mode=DR)
                if i % 2 == 0:
                    nc.vector.tensor_copy(KK[:, ts(i, NT)], kp[:])
                else:
                    nc.gpsimd.tensor_copy(KK[:, ts(i, NT)], kp[:])
            for j in range(s * (MCH // NSL), (s + 1) * (MCH // NSL)):
                vp = pp.tile([128, C], F32, tag="p")
                nc.tensor.matmul(vp[:], gui8[:, :, ts(j, MT)], wvt8[:, :, :],
                                 perf_mode=DR)
                # V' = gamma*V + gamma*bv folded into the drain
                if j % 2 == 0:
                    nc.vector.scalar_tensor_tensor(
                        VT2[:, j // 2, j % 2, :], vp[:], g128[:], bvg[:],
                        MUL, ADD)
                else:
                    nc.gpsimd.scalar_tensor_tensor(
                        VT2[:, j // 2, j % 2, :], vp[:], g128[:], bvg[:],
                        MUL, ADD)
        qp = pp.tile([128, NT], F32, tag="p")
        for t in range(CCH):
            nc.tensor.matmul(qp[:], wqt_f[:, t, :].bitcast(F32R),
                             src_sb[:, t, 0:NT].bitcast(F32R),
                             start=(t == 0), stop=(t == CCH - 1))
        nc.vector.tensor_copy(QQ[:, 0:NT], qp[:])

    # ---- attention main loop ----
    # PSUM budget: e_ps 2x2-bank + o_ps 2 + s_ps 1 + q_ps 1 = 8 banks.
    e_ps = ctx.enter_context(
        tc.tile_pool(name="e_psum" + sfx, bufs=2, space=bass.MemorySpace.PSUM))
    o_ps = ctx.enter_context(
        tc.tile_pool(name="o_psum" + sfx, bufs=2, space=bass.MemorySpace.PSUM))
    s_ps = ctx.enter_context(
        tc.tile_pool(name="s_psum" + sfx, bufs=1, space=bass.MemorySpace.PSUM))
    q_ps = ctx.enter_context(
        tc.tile_pool(name="q_psum" + sfx, bufs=1, space=bass.MemorySpace.PSUM))
    e_sb = ctx.enter_context(tc.tile_pool(name="e_sb" + sfx, bufs=4))
    fin = ctx.enter_context(tc.tile_pool(name="fin" + sfx, bufs=2))
    o_sb = ctx.enter_context(tc.tile_pool(name="o_sb" + sfx, bufs=4))

    out_r = out_d.rearrange("(t p) n -> t p n", p=128)
    NPAIR = MCH // 2  # 16 m-chunk pairs per query tile

    for i in range(NCH):
        o0 = o_ps.tile([128, NT], F32, tag="o")
        o1 = o_ps.tile([128, NT], F32, tag="o")
        sm = s_ps.tile([128, NT], F32, tag="s")

        def energy_pair(jj):
            # two m-chunks into one 2-bank PSUM tile -> single ACT exp drain
            ep = e_ps.tile([128, 2, NT], F32, tag="e")
            for h in range(2):
                j = 2 * jj + h
                b0 = CR * (j % 2)
                nc.tensor.matmul(ep[:, h, :], KK[b0:b0 + CR, ts(j, MT)],
                                 QQ[b0:b0 + CR, ts(i, NT)],
                                 start=True, stop=True, tile_position=(b0, 0))
            return ep

        ep = energy_pair(0)
        for jj in range(NPAIR):
            ee = e_sb.tile([128, 2, NT], FP8, tag="ee")
            nc.scalar.activation(ee[:], ep[:], EXP, bias=ebias[:])
            if jj + 1 < NPAIR:
                ep = energy_pair(jj + 1)  # keep PE one pair ahead of ACT
            if jj == 0 and i + 1 < NCH:
                # project Q for the next query tile in-loop (fp32r moving off
                # src_sb: full-rate rows, no conversion pass)
                qp = q_ps.tile([128, NT], F32, tag="q")
                for t in range(CCH):
                    nc.tensor.matmul(qp[:], wqt_f[:, t, :].bitcast(F32R),
                                     src_sb[:, t, ts(i + 1, NT)].bitcast(F32R),
                                     start=(t == 0), stop=(t == CCH - 1))
                nc.vector.tensor_copy(QQ[:, ts(i + 1, NT)], qp[:])
            first, last = jj == 0, jj == NPAIR - 1
            nc.tensor.matmul(o0[:], VT2[:, jj, :, 0:128], ee[:],
                             start=first, stop=last, perf_mode=DR)
            nc.tensor.matmul(o1[:], VT2[:, jj, :, 128:256], ee[:],
                             start=first, stop=last, perf_mode=DR)
            nc.tensor.matmul(sm[:], ones8[:], ee[:], start=first, stop=last,
                             perf_mode=DR)

        # out = o * (1/sum) + src  (gamma already folded into V')
        rsg = fin.tile([128, NT], F32, tag="rsg")
        nc.vector.reciprocal_approx_fast(rsg[:], sm[:])
        for t, op in enumerate((o0, o1)):
            ot = o_sb.tile([128, NT], F32, tag="ot")
            nc.vector.tensor_mul(ot[:], op[:], rsg[:])
            if i == NCH - 1:
                # last tile: residual add on DVE (shorter serial epilogue)
                nc.vector.tensor_add(ot[:], ot[:], src_sb[:, t, ts(i, NT)])
            else:
                nc.gpsimd.tensor_add(ot[:], ot[:], src_sb[:, t, ts(i, NT)])
            nc.sync.dma_start(out=out_r[t][:, ts(i, NT)], in_=ot[:])


_NC_CACHE = []


def _get_nc():
    if not _NC_CACHE:
        _NC_CACHE.append(build_kernel())
    return _NC_CACHE[0]


def make_in_maps(**inputs):
    f = lambda a: np.ascontiguousarray(np.asarray(a, dtype=np.float32))
    src = f(inputs["source"]).reshape(B, C, N)
    gui = f(inputs["guidance"]).reshape(B, C, N)
    shared = {
        "Wq": f(inputs["Wq"]),
        "Wk": f(inputs["Wk"]),
        "Wv": f(inputs["Wv"]),
        "bv": f(inputs["bv"]),
        "gamma": f(inputs["gamma"]),
    }
    return [dict(source=src[b], guidance=gui[b], **shared) for b in range(B)]


def kernel(**inputs) -> np.ndarray:
    nc = _get_nc()
    res = run_bass_kernel_spmd(nc, make_in_maps(**inputs),
                               core_ids=list(range(N_CORES)))
    out = np.stack([res.results[b]["out"] for b in range(B)])
    return out.reshape(B, C, H, W).astype(np.float32)
